# revision 2
# baseline (speedup 1.0000x reference)
"""Nearest-E8 quantizer, v4: rebalanced engines + q-space smalls.

Same math as v3 (see kernel.py docstring), restructured:
  - parities as sign-multiplies: q0 = (-1)^parity(Sr) via bit0<<31 trick,
    qx from xor-reduce bit31, q1 = q0*qx on Pool.
  - margin m = (2-Sa) + e1*(1-q1) - e0*(1-q0), e0 = 1/2-ma+eps, e1 = mina+eps
  - branch shift Gh = c0 ? p0*e0 : -p1*e1 ; Hh = c1/2
  - y = round(x + sign(d0)*Gh - Hh) + Hh, bf16 out
Engine assignment is table-driven; lumpy ops can alternate engines by tile
parity for fractional balance.
"""

import numpy as np

from concourse import bacc
import concourse.mybir as mybir
from concourse.alu_op_type import AluOpType as op
from concourse.bass_utils import run_bass_kernel_spmd
from concourse.tile import TileContext

N_POINTS = 8388608
N_CORES = 8
SHARD = N_POINTS // N_CORES

MAGIC = 12582912.0
EPS = float(2.0 ** -20)
F32 = mybir.dt.float32
BF16 = mybir.dt.bfloat16
U32 = mybir.dt.uint32
X = mybir.AxisListType.X
CP = mybir.ActivationFunctionType.Copy

# engine per op; value "v"=DVE, "g"=Pool, "a"=ACT; tuple = alternate by tile parity
CFG = {
    "tf": 256,
    "bufs": 4,
    "par": "ta",       # "sr": add-reduce r0 + parity-extract; "ta": xor-reduce (x+M)
    "r0": "v",         # ts round
    "d0": "g",         # TT sub
    "v": "g",          # TT add (in-place over xt)
    "u": "g",          # TT bcast sub (in-place)
    "w": "v",          # ts 2-op round (in-place)
    "y": "g",          # +H bcast, bf16 out
    "q1": "v",         # small q0*qx   (Pool smalls are ~1.7us on HW: keep on DVE)
    "bb": "v",         # smalls b1,b2 (e*q mults)
    "cc": "v",         # smalls c1t,c2t (subs)
    "mm": "v",         # smalls m-chain (2 TTs)
    "merge": False,    # fuse ta|d0 xor-reduce, b1|b2, c1t|c2t into single ops
}
ENGINES = CFG  # back-compat for test.py


def _imm_u32(v):
    return mybir.ImmediateValue(dtype=U32, value=v)


def _ts_u32(eng, out, in0, s0, s1, op0, op1):
    return eng.add_instruction(
        mybir.InstTensorScalarPtr(
            name=eng.bass.get_next_instruction_name(),
            op0=op0, op1=op1,
            ins=[eng.lower_ap(in0), _imm_u32(s0), _imm_u32(s1)],
            outs=[eng.lower_ap(out)],
        )
    )


def _stt_u32(eng, out, in0, scalar_int, in1, op0, op1):
    return eng.add_instruction(
        mybir.InstTensorScalarPtr(
            name=eng.bass.get_next_instruction_name(),
            is_scalar_tensor_tensor=True,
            op0=op0, op1=op1,
            ins=[eng.lower_ap(in0), _imm_u32(scalar_int), eng.lower_ap(in1)],
            outs=[eng.lower_ap(out)],
        )
    )


def _eng(nc, key, t):
    e = CFG[key]
    if isinstance(e, tuple):
        e = e[t % len(e)]
    return {"v": nc.vector, "g": nc.gpsimd, "a": nc.scalar}[e]


def _emit_front1(nc, pools, xd, t, tf):
    P = 128
    pts = P * tf
    FE = tf * 8
    stream, work, small = pools
    s = t * pts
    xt = stream.tile([P, FE], F32, tag="xt")
    nc.sync.dma_start(out=xt[:], in_=xd[s:s + pts, :].rearrange("(p f) c -> p (f c)", p=P))
    re = CFG["r0"]
    re_t = re[t % len(re)] if isinstance(re, tuple) else re
    if CFG["par"] == "ta" and CFG.get("merge"):
        # one [P, 2FE] tile: [ta | d0]; r0 transient in d0's half
        big = work.tile([P, 2 * FE], F32, tag="big")
        ta, d0 = big[:, :FE], big[:, FE:]
        e = {"v": nc.vector, "g": nc.gpsimd}[re_t if re_t != "a" else "v"]
        e.tensor_scalar(ta, xt[:], MAGIC, None, op0=op.add)
        e.tensor_scalar(d0, ta, MAGIC, None, op0=op.subtract)  # r0 in d0 slot
        _eng(nc, "d0", t).tensor_tensor(d0, xt[:], d0, op.subtract)
        return dict(t=t, xt=xt, big=big, r0=None, d0v=d0)
    r0 = work.tile([P, FE], F32, tag="r0")
    d0 = work.tile([P, FE], F32, tag="d0")
    ta = None
    if CFG["par"] == "ta":
        # tA = x + MAGIC (bit0 = parity of round(x)); r0 = tA - MAGIC
        ta = work.tile([P, FE], F32, tag="ta")
        if re_t == "a":
            nc.scalar.activation(ta[:], xt[:], CP, bias=MAGIC)
            nc.scalar.activation(r0[:], ta[:], CP, bias=-MAGIC)
        else:
            e = {"v": nc.vector, "g": nc.gpsimd}[re_t]
            e.tensor_scalar(ta[:], xt[:], MAGIC, None, op0=op.add)
            e.tensor_scalar(r0[:], ta[:], MAGIC, None, op0=op.subtract)
    else:
        if re_t == "a":
            nc.scalar.activation(r0[:], xt[:], CP, bias=MAGIC)
            nc.scalar.activation(r0[:], r0[:], CP, bias=-MAGIC)
        else:
            e = {"v": nc.vector, "g": nc.gpsimd}[re_t]
            e.tensor_scalar(r0[:], xt[:], MAGIC, MAGIC, op0=op.add, op1=op.subtract)
    _eng(nc, "d0", t).tensor_tensor(d0[:], xt[:], r0[:], op.subtract)
    return dict(t=t, xt=xt, r0=r0, d0=d0, ta=ta, d0v=None)


def _emit_front2(nc, pools, st, tf):
    """Reduces + q-space smalls."""
    P = 128
    stream, work, small = pools
    t = st["t"]
    V, G, A = nc.vector, nc.gpsimd, nc.scalar
    Eq = {"v": V, "g": G}

    if CFG["par"] == "ta" and CFG.get("merge"):
        return _emit_front2_merged(nc, pools, st, tf)

    r0, d0 = st["r0"], st["d0"]
    d03 = d0[:].rearrange("p (t c) -> p t c", c=8)
    r03 = r0[:].rearrange("p (t c) -> p t c", c=8)
    d0u3 = d0[:].bitcast(U32).rearrange("p (t c) -> p t c", c=8)

    NS = 10
    arena = small.tile([P, NS * tf], F32, tag="arena")
    sl = lambda i: arena[:, i * tf:(i + 1) * tf]
    Sr, xr, Sa, ma, mina = sl(0), sl(1), sl(2), sl(3), sl(4)
    q1, e0, e1, Gh, Hh = sl(5), sl(6), sl(7), sl(8), sl(9)

    if CFG["par"] == "ta":
        ta3 = st["ta"][:].bitcast(U32).rearrange("p (t c) -> p t c", c=8)
        V.tensor_reduce(Sr.bitcast(U32), ta3, axis=X, op=op.bitwise_xor)
    else:
        V.tensor_reduce(Sr, r03, axis=X, op=op.add)
    V.tensor_reduce(xr.bitcast(U32), d0u3, axis=X, op=op.bitwise_xor)
    V.tensor_reduce(Sa, d03, axis=X, op=op.add, apply_absolute_value=True)
    V.tensor_reduce(ma, d03, axis=X, op=op.max, apply_absolute_value=True)
    V.tensor_reduce(mina, d03, axis=X, op=op.min, apply_absolute_value=True)

    # q0 = (-1)^parity(sum r0): from xor(tA) bit0, or from (Sr+MAGIC) bit0
    q0 = Sr  # in-place chain over Sr's slot
    if CFG["par"] != "ta":
        V.tensor_scalar(q0, Sr, MAGIC, None, op0=op.add)
    _ts_u32(V, q0.bitcast(U32), q0.bitcast(U32), 31, 0x3F800000,
            op.logical_shift_left, op.bitwise_xor)
    # qx = (-1)^parity(#neg): bit31 of xor-reduce
    qx = xr
    _ts_u32(V, qx.bitcast(U32), xr.bitcast(U32), 0x80000000, 0x3F800000,
            op.bitwise_and, op.bitwise_xor)
    # q1 = q0*qx
    Eq[CFG["q1"]].tensor_tensor(q1, q0, qx, op.mult)
    # e0 = 1/2 - ma + eps ; e1 = mina + eps ; s2 = 2 - Sa (into Sa's slot)
    A.activation(e0, ma, CP, bias=0.5 + EPS, scale=-1.0)
    A.activation(e1, mina, CP, bias=EPS)
    s2 = Sa
    A.activation(s2, Sa, CP, bias=2.0, scale=-1.0)
    # b1 = e1*q1, b2 = e0*q0 (Pool); c1t = e1-b1 = 2*p1*e1, c2t = e0-b2 = 2*p0*e0
    b1, b2 = ma, mina  # reuse slots (ma/mina consumed by e0/e1)
    Eb = Eq[CFG["bb"]]
    Eb.tensor_tensor(b1, e1, q1, op.mult)
    Eb.tensor_tensor(b2, e0, q0, op.mult)
    c1t, c2t = b1, b2
    Ec = Eq[CFG["cc"]]
    Ec.tensor_tensor(c1t, e1, b1, op.subtract)
    Ec.tensor_tensor(c2t, e0, b2, op.subtract)
    # Gh candidates BEFORE m clobbers anything: Gh0 = c2t/2, g1 = -c1t/2
    A.activation(Gh, c2t, CP, scale=0.5)
    g1 = q1  # q1 slot dead after b1
    A.activation(g1, c1t, CP, scale=-0.5)
    # m = (s2 - c2t) + c1t  (into s2's slot); c0f = m >= 0 (in place)
    m = s2
    Em = Eq[CFG["mm"]]
    Em.tensor_tensor(m, s2, c2t, op.subtract)
    Em.tensor_tensor(m, m, c1t, op.add)
    c0f = m
    V.tensor_scalar(c0f, m, 0.0, None, op0=op.is_ge)
    # Hh = 0.5*(1-c0f); then Gh = branch1 ? g1 : Gh0 (Hh doubles as the mask)
    A.activation(Hh, c0f, CP, bias=0.5, scale=-0.5)
    V.copy_predicated(Gh, Hh.bitcast(U32), g1)

    st["Gh"] = Gh
    st["Hh"] = Hh
    return st


def _emit_front2_merged(nc, pools, st, tf):
    """Merged variant: one xor-reduce over [ta|d0]; fused b/c smalls.

    Arena slots: 0 q0raw->q0, 1 xraw->qx->q1, 2 Sa->s2->m->c0f,
    3 ma->b2->c2t, 4 mina->b1->c1t, 5 e0, 6 e1, 7 tmp, 8 g1, 9 Gh, 10 Hh.
    Pairings: b-mult out (3,4) = in0 (5,6)=(e0|e1) * in1 (0,1)=(q0|q1);
    c-sub out (3,4) = in0 (5,6) - in1 (3,4).
    => slot3 = e0*q0 = b2, c2t = e0-b2 ; slot4 = e1*q1 = b1, c1t = e1-b1.
    """
    P = 128
    stream, work, small = pools
    t = st["t"]
    V, G, A = nc.vector, nc.gpsimd, nc.scalar
    Eq = {"v": V, "g": G}
    big = st["big"]
    FE = tf * 8
    d0 = st["d0v"]
    d03 = d0.rearrange("p (t c) -> p t c", c=8)

    NS = 11
    arena = small.tile([P, NS * tf], F32, tag="arena")
    sl = lambda i: arena[:, i * tf:(i + 1) * tf]

    big4 = big[:].bitcast(U32).rearrange("p (k t c) -> p k t c", k=2, c=8)
    xo2 = arena[:, 0:2 * tf].bitcast(U32).rearrange("p (k t) -> p k t", k=2)
    V.tensor_reduce(xo2, big4, axis=X, op=op.bitwise_xor)
    Sa, ma, mina = sl(2), sl(3), sl(4)
    V.tensor_reduce(Sa, d03, axis=X, op=op.add, apply_absolute_value=True)
    V.tensor_reduce(ma, d03, axis=X, op=op.max, apply_absolute_value=True)
    V.tensor_reduce(mina, d03, axis=X, op=op.min, apply_absolute_value=True)

    q0, qx = sl(0), sl(1)
    _ts_u32(V, q0.bitcast(U32), q0.bitcast(U32), 31, 0x3F800000,
            op.logical_shift_left, op.bitwise_xor)
    _ts_u32(V, qx.bitcast(U32), qx.bitcast(U32), 0x80000000, 0x3F800000,
            op.bitwise_and, op.bitwise_xor)
    q1 = qx  # in-place over qx -> (q0|q1) adjacent
    Eq[CFG["q1"]].tensor_tensor(q1, q0, qx, op.mult)

    e0, e1 = sl(5), sl(6)
    A.activation(e0, ma, CP, bias=0.5 + EPS, scale=-1.0)
    A.activation(e1, mina, CP, bias=EPS)
    s2 = Sa
    A.activation(s2, Sa, CP, bias=2.0, scale=-1.0)

    # fused b-mult: (3,4) = (e0|e1) * (q0|q1) ; fused c-sub: (3,4) = (e0|e1) - (3,4)
    ee = arena[:, 5 * tf:7 * tf]
    qq = arena[:, 0:2 * tf]
    bb = arena[:, 3 * tf:5 * tf]
    Eq[CFG["bb"]].tensor_tensor(bb, ee, qq, op.mult)
    Eq[CFG["cc"]].tensor_tensor(bb, ee, bb, op.subtract)
    c2t, c1t = sl(3), sl(4)

    Gh, Hh, g1 = sl(9), sl(10), sl(8)
    A.activation(Gh, c2t, CP, scale=0.5)
    A.activation(g1, c1t, CP, scale=-0.5)

    m = s2
    Em = Eq[CFG["mm"]]
    Em.tensor_tensor(m, s2, c2t, op.subtract)
    Em.tensor_tensor(m, m, c1t, op.add)
    c0f = m
    V.tensor_scalar(c0f, m, 0.0, None, op0=op.is_ge)
    A.activation(Hh, c0f, CP, bias=0.5, scale=-0.5)
    V.copy_predicated(Gh, Hh.bitcast(U32), g1)

    st["Gh"] = Gh
    st["Hh"] = Hh
    return st


def _emit_back1(nc, pools, st, tf):
    P = 128
    t = st["t"]
    xt, Gh = st["xt"], st["Gh"]
    d0ap = st["d0v"] if st.get("d0v") is not None else st["d0"][:]
    d0u3 = d0ap.bitcast(U32).rearrange("p (t c) -> p t c", c=8)
    Gh_b = Gh.bitcast(U32).unsqueeze(2).broadcast_to([P, tf, 8])
    # g = (d0 & sign) ^ Gh  (in place over d0)  [DVE only: stt]
    _stt_u32(nc.vector, d0u3, d0u3, 0x80000000, Gh_b, op.bitwise_and, op.bitwise_xor)
    # v = x + g (in place over xt)
    _eng(nc, "v", t).tensor_tensor(xt[:], xt[:], d0ap, op.add)
    return st


def _emit_back2(nc, pools, yd, st, tf):
    P = 128
    pts = P * tf
    FE = tf * 8
    stream, work, small = pools
    t = st["t"]
    xt, Hh = st["xt"], st["Hh"]
    s = t * pts
    y_rows = yd[s:s + pts, :].rearrange("(p f) c -> p (f c)", p=P)
    H_b = Hh.unsqueeze(2).broadcast_to([P, tf, 8])
    xt3 = xt[:].rearrange("p (t c) -> p t c", c=8)
    # u = v - H (in place)
    _eng(nc, "u", t).tensor_tensor(xt3, xt3, H_b, op.subtract)
    # w = round(u) (in place)
    we = CFG["w"]
    if we == "a":
        nc.scalar.activation(xt[:], xt[:], CP, bias=MAGIC)
        nc.scalar.activation(xt[:], xt[:], CP, bias=-MAGIC)
    else:
        _eng(nc, "w", t).tensor_scalar(xt[:], xt[:], MAGIC, MAGIC, op0=op.add, op1=op.subtract)
    # y = w + H -> bf16
    yt = stream.tile([P, FE], BF16, tag="yt")
    yt3 = yt[:].rearrange("p (t c) -> p t c", c=8)
    ye = CFG["y"]
    ye_t = ye[t % len(ye)] if isinstance(ye, tuple) else ye
    if ye_t == "v":
        nc.vector.scalar_tensor_tensor(yt3, H_b, 1.0, xt3, op0=op.mult, op1=op.add)
    else:
        nc.gpsimd.tensor_tensor(yt3, xt3, H_b, op.add)
    nc.sync.dma_start(out=y_rows, in_=yt[:])


def build_nc(shard=SHARD, tf=None, reps=1, cfg=None):
    if cfg:
        CFG.update(cfg)
    P = 128
    tf = tf or CFG["tf"]
    pts = P * tf
    assert shard % pts == 0
    ntiles = shard // pts

    nc = bacc.Bacc("TRN2", target_bir_lowering=False, debug=False, num_devices=N_CORES)
    xd = nc.declare_dram_parameter("x", [shard, 8], F32, isOutput=False)
    yd = nc.declare_dram_parameter("y", [shard, 8], BF16, isOutput=True)

    with TileContext(nc) as tc:
        with (
            tc.tile_pool(name="stream", bufs=CFG["bufs"]) as stream,
            tc.tile_pool(name="work", bufs=CFG["bufs"]) as work,
            tc.tile_pool(name="small", bufs=CFG["bufs"]) as small,
        ):
            for _ in range(reps):
                pools = (stream, work, small)
                stages = [None, None, None]
                for t in range(ntiles + 3):
                    nxt = _emit_front1(nc, pools, xd, t, tf) if t < ntiles else None
                    if stages[2] is not None:
                        _emit_back2(nc, pools, yd, stages[2], tf)
                    stages[2] = _emit_back1(nc, pools, stages[1], tf) if stages[1] is not None else None
                    stages[1] = _emit_front2(nc, pools, stages[0], tf) if stages[0] is not None else None
                    stages[0] = nxt
    nc.finalize()
    return nc


_BUILD_CACHE = {}


def _get_nc(shard, tf):
    key = (shard, tf)
    if key not in _BUILD_CACHE:
        _BUILD_CACHE[key] = build_nc(shard, tf)
    return _BUILD_CACHE[key]


def kernel(x: np.ndarray) -> np.ndarray:
    x = np.ascontiguousarray(x, dtype=np.float32)
    n = x.shape[0]
    shard = n // N_CORES
    tf = CFG["tf"]
    while shard % (128 * tf) != 0:
        tf //= 2
    nc = _get_nc(shard, tf)
    in_maps = [{"x": x[i * shard:(i + 1) * shard]} for i in range(N_CORES)]
    res = run_bass_kernel_spmd(nc, in_maps, list(range(N_CORES))).results
    out = np.concatenate([res[i]["y"] for i in range(N_CORES)], axis=0)
    return np.ascontiguousarray(out.astype(np.float32))


# revision 3
# speedup vs baseline: 1.0177x; 1.0177x over previous
"""Nearest-E8 quantizer, v4: rebalanced engines + q-space smalls.

Same math as v3 (see kernel.py docstring), restructured:
  - parities as sign-multiplies: q0 = (-1)^parity(Sr) via bit0<<31 trick,
    qx from xor-reduce bit31, q1 = q0*qx on Pool.
  - margin m = (2-Sa) + e1*(1-q1) - e0*(1-q0), e0 = 1/2-ma+eps, e1 = mina+eps
  - branch shift Gh = c0 ? p0*e0 : -p1*e1 ; Hh = c1/2
  - y = round(x + sign(d0)*Gh - Hh) + Hh, bf16 out
Engine assignment is table-driven; lumpy ops can alternate engines by tile
parity for fractional balance.
"""

import numpy as np

from concourse import bacc
import concourse.mybir as mybir
from concourse.alu_op_type import AluOpType as op
from concourse.bass_utils import run_bass_kernel_spmd
from concourse.tile import TileContext

N_POINTS = 8388608
N_CORES = 8
SHARD = N_POINTS // N_CORES

MAGIC = 12582912.0
EPS = float(2.0 ** -20)
F32 = mybir.dt.float32
BF16 = mybir.dt.bfloat16
U32 = mybir.dt.uint32
X = mybir.AxisListType.X
CP = mybir.ActivationFunctionType.Copy

# engine per op; value "v"=DVE, "g"=Pool, "a"=ACT; tuple = alternate by tile parity
CFG = {
    "tf": 256,
    "bufs": 4,
    "par": "sr",       # "sr": add-reduce r0 + parity-extract; "ta": xor-reduce (x+M)
    "r0": "v",         # ts round
    "d0": "g",         # TT sub
    "v": "g",          # TT add (in-place over xt)
    "u": "g",          # TT bcast sub (in-place)
    "w": "v",          # ts 2-op round (in-place)
    "y": "g",          # +H bcast, bf16 out
    "q1": "v",         # small q0*qx   (Pool smalls are ~1.7us on HW: keep on DVE)
    "bb": "v",         # smalls b1,b2 (e*q mults)
    "cc": "v",         # smalls c1t,c2t (subs)
    "mm": "v",         # smalls m-chain (2 TTs)
    "merge": False,    # fuse ta|d0 xor-reduce, b1|b2, c1t|c2t into single ops
}
ENGINES = CFG  # back-compat for test.py


def _imm_u32(v):
    return mybir.ImmediateValue(dtype=U32, value=v)


def _ts_u32(eng, out, in0, s0, s1, op0, op1):
    return eng.add_instruction(
        mybir.InstTensorScalarPtr(
            name=eng.bass.get_next_instruction_name(),
            op0=op0, op1=op1,
            ins=[eng.lower_ap(in0), _imm_u32(s0), _imm_u32(s1)],
            outs=[eng.lower_ap(out)],
        )
    )


def _stt_u32(eng, out, in0, scalar_int, in1, op0, op1):
    return eng.add_instruction(
        mybir.InstTensorScalarPtr(
            name=eng.bass.get_next_instruction_name(),
            is_scalar_tensor_tensor=True,
            op0=op0, op1=op1,
            ins=[eng.lower_ap(in0), _imm_u32(scalar_int), eng.lower_ap(in1)],
            outs=[eng.lower_ap(out)],
        )
    )


def _eng(nc, key, t):
    e = CFG[key]
    if isinstance(e, tuple):
        e = e[t % len(e)]
    return {"v": nc.vector, "g": nc.gpsimd, "a": nc.scalar}[e]


def _emit_front1(nc, pools, xd, t, tf):
    P = 128
    pts = P * tf
    FE = tf * 8
    stream, work, small = pools
    s = t * pts
    xt = stream.tile([P, FE], F32, tag="xt")
    nc.sync.dma_start(out=xt[:], in_=xd[s:s + pts, :].rearrange("(p f) c -> p (f c)", p=P))
    re = CFG["r0"]
    re_t = re[t % len(re)] if isinstance(re, tuple) else re
    if CFG["par"] == "ta" and CFG.get("merge"):
        # one [P, 2FE] tile: [ta | d0]; r0 transient in d0's half
        big = work.tile([P, 2 * FE], F32, tag="big")
        ta, d0 = big[:, :FE], big[:, FE:]
        e = {"v": nc.vector, "g": nc.gpsimd}[re_t if re_t != "a" else "v"]
        e.tensor_scalar(ta, xt[:], MAGIC, None, op0=op.add)
        e.tensor_scalar(d0, ta, MAGIC, None, op0=op.subtract)  # r0 in d0 slot
        _eng(nc, "d0", t).tensor_tensor(d0, xt[:], d0, op.subtract)
        return dict(t=t, xt=xt, big=big, r0=None, d0v=d0)
    r0 = work.tile([P, FE], F32, tag="r0")
    d0 = work.tile([P, FE], F32, tag="d0")
    ta = None
    if CFG["par"] == "ta":
        # tA = x + MAGIC (bit0 = parity of round(x)); r0 = tA - MAGIC
        ta = work.tile([P, FE], F32, tag="ta")
        if re_t == "a":
            nc.scalar.activation(ta[:], xt[:], CP, bias=MAGIC)
            nc.scalar.activation(r0[:], ta[:], CP, bias=-MAGIC)
        else:
            e = {"v": nc.vector, "g": nc.gpsimd}[re_t]
            e.tensor_scalar(ta[:], xt[:], MAGIC, None, op0=op.add)
            e.tensor_scalar(r0[:], ta[:], MAGIC, None, op0=op.subtract)
    else:
        if re_t == "a":
            nc.scalar.activation(r0[:], xt[:], CP, bias=MAGIC)
            nc.scalar.activation(r0[:], r0[:], CP, bias=-MAGIC)
        else:
            e = {"v": nc.vector, "g": nc.gpsimd}[re_t]
            e.tensor_scalar(r0[:], xt[:], MAGIC, MAGIC, op0=op.add, op1=op.subtract)
    _eng(nc, "d0", t).tensor_tensor(d0[:], xt[:], r0[:], op.subtract)
    return dict(t=t, xt=xt, r0=r0, d0=d0, ta=ta, d0v=None)


def _emit_front2(nc, pools, st, tf):
    """Reduces + q-space smalls."""
    P = 128
    stream, work, small = pools
    t = st["t"]
    V, G, A = nc.vector, nc.gpsimd, nc.scalar
    Eq = {"v": V, "g": G}

    if CFG["par"] == "ta" and CFG.get("merge"):
        return _emit_front2_merged(nc, pools, st, tf)

    r0, d0 = st["r0"], st["d0"]
    d03 = d0[:].rearrange("p (t c) -> p t c", c=8)
    r03 = r0[:].rearrange("p (t c) -> p t c", c=8)
    d0u3 = d0[:].bitcast(U32).rearrange("p (t c) -> p t c", c=8)

    NS = 10
    arena = small.tile([P, NS * tf], F32, tag="arena")
    sl = lambda i: arena[:, i * tf:(i + 1) * tf]
    Sr, xr, Sa, ma, mina = sl(0), sl(1), sl(2), sl(3), sl(4)
    q1, e0, e1, Gh, Hh = sl(5), sl(6), sl(7), sl(8), sl(9)

    if CFG["par"] == "ta":
        ta3 = st["ta"][:].bitcast(U32).rearrange("p (t c) -> p t c", c=8)
        V.tensor_reduce(Sr.bitcast(U32), ta3, axis=X, op=op.bitwise_xor)
    else:
        V.tensor_reduce(Sr, r03, axis=X, op=op.add)
    V.tensor_reduce(xr.bitcast(U32), d0u3, axis=X, op=op.bitwise_xor)
    V.tensor_reduce(Sa, d03, axis=X, op=op.add, apply_absolute_value=True)
    V.tensor_reduce(ma, d03, axis=X, op=op.max, apply_absolute_value=True)
    V.tensor_reduce(mina, d03, axis=X, op=op.min, apply_absolute_value=True)

    # q0 = (-1)^parity(sum r0): from xor(tA) bit0, or from (Sr+MAGIC) bit0
    q0 = Sr  # in-place chain over Sr's slot
    if CFG["par"] != "ta":
        V.tensor_scalar(q0, Sr, MAGIC, None, op0=op.add)
    _ts_u32(V, q0.bitcast(U32), q0.bitcast(U32), 31, 0x3F800000,
            op.logical_shift_left, op.bitwise_xor)
    # qx = (-1)^parity(#neg): bit31 of xor-reduce
    qx = xr
    _ts_u32(V, qx.bitcast(U32), xr.bitcast(U32), 0x80000000, 0x3F800000,
            op.bitwise_and, op.bitwise_xor)
    # q1 = q0*qx
    Eq[CFG["q1"]].tensor_tensor(q1, q0, qx, op.mult)
    # e0 = 1/2 - ma + eps ; e1 = mina + eps ; s2 = 2 - Sa (into Sa's slot)
    A.activation(e0, ma, CP, bias=0.5 + EPS, scale=-1.0)
    A.activation(e1, mina, CP, bias=EPS)
    s2 = Sa
    A.activation(s2, Sa, CP, bias=2.0, scale=-1.0)
    # b1 = e1*q1, b2 = e0*q0 (Pool); c1t = e1-b1 = 2*p1*e1, c2t = e0-b2 = 2*p0*e0
    b1, b2 = ma, mina  # reuse slots (ma/mina consumed by e0/e1)
    Eb = Eq[CFG["bb"]]
    Eb.tensor_tensor(b1, e1, q1, op.mult)
    Eb.tensor_tensor(b2, e0, q0, op.mult)
    c1t, c2t = b1, b2
    Ec = Eq[CFG["cc"]]
    Ec.tensor_tensor(c1t, e1, b1, op.subtract)
    Ec.tensor_tensor(c2t, e0, b2, op.subtract)
    # Gh candidates BEFORE m clobbers anything: Gh0 = c2t/2, g1 = -c1t/2
    A.activation(Gh, c2t, CP, scale=0.5)
    g1 = q1  # q1 slot dead after b1
    A.activation(g1, c1t, CP, scale=-0.5)
    # m = (s2 - c2t) + c1t  (into s2's slot); c0f = m >= 0 (in place)
    m = s2
    Em = Eq[CFG["mm"]]
    Em.tensor_tensor(m, s2, c2t, op.subtract)
    Em.tensor_tensor(m, m, c1t, op.add)
    c0f = m
    V.tensor_scalar(c0f, m, 0.0, None, op0=op.is_ge)
    # Hh = 0.5*(1-c0f); then Gh = branch1 ? g1 : Gh0 (Hh doubles as the mask)
    A.activation(Hh, c0f, CP, bias=0.5, scale=-0.5)
    V.copy_predicated(Gh, Hh.bitcast(U32), g1)

    st["Gh"] = Gh
    st["Hh"] = Hh
    return st


def _emit_front2_merged(nc, pools, st, tf):
    """Merged variant: one xor-reduce over [ta|d0]; fused b/c smalls.

    Arena slots: 0 q0raw->q0, 1 xraw->qx->q1, 2 Sa->s2->m->c0f,
    3 ma->b2->c2t, 4 mina->b1->c1t, 5 e0, 6 e1, 7 tmp, 8 g1, 9 Gh, 10 Hh.
    Pairings: b-mult out (3,4) = in0 (5,6)=(e0|e1) * in1 (0,1)=(q0|q1);
    c-sub out (3,4) = in0 (5,6) - in1 (3,4).
    => slot3 = e0*q0 = b2, c2t = e0-b2 ; slot4 = e1*q1 = b1, c1t = e1-b1.
    """
    P = 128
    stream, work, small = pools
    t = st["t"]
    V, G, A = nc.vector, nc.gpsimd, nc.scalar
    Eq = {"v": V, "g": G}
    big = st["big"]
    FE = tf * 8
    d0 = st["d0v"]
    d03 = d0.rearrange("p (t c) -> p t c", c=8)

    NS = 11
    arena = small.tile([P, NS * tf], F32, tag="arena")
    sl = lambda i: arena[:, i * tf:(i + 1) * tf]

    big4 = big[:].bitcast(U32).rearrange("p (k t c) -> p k t c", k=2, c=8)
    xo2 = arena[:, 0:2 * tf].bitcast(U32).rearrange("p (k t) -> p k t", k=2)
    V.tensor_reduce(xo2, big4, axis=X, op=op.bitwise_xor)
    Sa, ma, mina = sl(2), sl(3), sl(4)
    V.tensor_reduce(Sa, d03, axis=X, op=op.add, apply_absolute_value=True)
    V.tensor_reduce(ma, d03, axis=X, op=op.max, apply_absolute_value=True)
    V.tensor_reduce(mina, d03, axis=X, op=op.min, apply_absolute_value=True)

    q0, qx = sl(0), sl(1)
    _ts_u32(V, q0.bitcast(U32), q0.bitcast(U32), 31, 0x3F800000,
            op.logical_shift_left, op.bitwise_xor)
    _ts_u32(V, qx.bitcast(U32), qx.bitcast(U32), 0x80000000, 0x3F800000,
            op.bitwise_and, op.bitwise_xor)
    q1 = qx  # in-place over qx -> (q0|q1) adjacent
    Eq[CFG["q1"]].tensor_tensor(q1, q0, qx, op.mult)

    e0, e1 = sl(5), sl(6)
    A.activation(e0, ma, CP, bias=0.5 + EPS, scale=-1.0)
    A.activation(e1, mina, CP, bias=EPS)
    s2 = Sa
    A.activation(s2, Sa, CP, bias=2.0, scale=-1.0)

    # fused b-mult: (3,4) = (e0|e1) * (q0|q1) ; fused c-sub: (3,4) = (e0|e1) - (3,4)
    ee = arena[:, 5 * tf:7 * tf]
    qq = arena[:, 0:2 * tf]
    bb = arena[:, 3 * tf:5 * tf]
    Eq[CFG["bb"]].tensor_tensor(bb, ee, qq, op.mult)
    Eq[CFG["cc"]].tensor_tensor(bb, ee, bb, op.subtract)
    c2t, c1t = sl(3), sl(4)

    Gh, Hh, g1 = sl(9), sl(10), sl(8)
    A.activation(Gh, c2t, CP, scale=0.5)
    A.activation(g1, c1t, CP, scale=-0.5)

    m = s2
    Em = Eq[CFG["mm"]]
    Em.tensor_tensor(m, s2, c2t, op.subtract)
    Em.tensor_tensor(m, m, c1t, op.add)
    c0f = m
    V.tensor_scalar(c0f, m, 0.0, None, op0=op.is_ge)
    A.activation(Hh, c0f, CP, bias=0.5, scale=-0.5)
    V.copy_predicated(Gh, Hh.bitcast(U32), g1)

    st["Gh"] = Gh
    st["Hh"] = Hh
    return st


def _emit_back1(nc, pools, st, tf):
    P = 128
    t = st["t"]
    xt, Gh = st["xt"], st["Gh"]
    d0ap = st["d0v"] if st.get("d0v") is not None else st["d0"][:]
    d0u3 = d0ap.bitcast(U32).rearrange("p (t c) -> p t c", c=8)
    Gh_b = Gh.bitcast(U32).unsqueeze(2).broadcast_to([P, tf, 8])
    # g = (d0 & sign) ^ Gh  (in place over d0)  [DVE only: stt]
    _stt_u32(nc.vector, d0u3, d0u3, 0x80000000, Gh_b, op.bitwise_and, op.bitwise_xor)
    # v = x + g (in place over xt)
    _eng(nc, "v", t).tensor_tensor(xt[:], xt[:], d0ap, op.add)
    return st


def _emit_back2(nc, pools, yd, st, tf):
    P = 128
    pts = P * tf
    FE = tf * 8
    stream, work, small = pools
    t = st["t"]
    xt, Hh = st["xt"], st["Hh"]
    s = t * pts
    y_rows = yd[s:s + pts, :].rearrange("(p f) c -> p (f c)", p=P)
    H_b = Hh.unsqueeze(2).broadcast_to([P, tf, 8])
    xt3 = xt[:].rearrange("p (t c) -> p t c", c=8)
    # u = v - H (in place)
    _eng(nc, "u", t).tensor_tensor(xt3, xt3, H_b, op.subtract)
    # w = round(u) (in place)
    we = CFG["w"]
    if we == "a":
        nc.scalar.activation(xt[:], xt[:], CP, bias=MAGIC)
        nc.scalar.activation(xt[:], xt[:], CP, bias=-MAGIC)
    else:
        _eng(nc, "w", t).tensor_scalar(xt[:], xt[:], MAGIC, MAGIC, op0=op.add, op1=op.subtract)
    # y = w + H -> bf16
    yt = stream.tile([P, FE], BF16, tag="yt")
    yt3 = yt[:].rearrange("p (t c) -> p t c", c=8)
    ye = CFG["y"]
    ye_t = ye[t % len(ye)] if isinstance(ye, tuple) else ye
    if ye_t == "v":
        nc.vector.scalar_tensor_tensor(yt3, H_b, 1.0, xt3, op0=op.mult, op1=op.add)
    else:
        nc.gpsimd.tensor_tensor(yt3, xt3, H_b, op.add)
    nc.sync.dma_start(out=y_rows, in_=yt[:])


def build_nc(shard=SHARD, tf=None, reps=1, cfg=None):
    if cfg:
        CFG.update(cfg)
    P = 128
    tf = tf or CFG["tf"]
    pts = P * tf
    assert shard % pts == 0
    ntiles = shard // pts

    nc = bacc.Bacc("TRN2", target_bir_lowering=False, debug=False, num_devices=N_CORES)
    xd = nc.declare_dram_parameter("x", [shard, 8], F32, isOutput=False)
    yd = nc.declare_dram_parameter("y", [shard, 8], BF16, isOutput=True)

    with TileContext(nc) as tc:
        with (
            tc.tile_pool(name="stream", bufs=CFG["bufs"]) as stream,
            tc.tile_pool(name="work", bufs=CFG["bufs"]) as work,
            tc.tile_pool(name="small", bufs=CFG["bufs"]) as small,
        ):
            for _ in range(reps):
                pools = (stream, work, small)
                stages = [None, None, None]
                for t in range(ntiles + 3):
                    nxt = _emit_front1(nc, pools, xd, t, tf) if t < ntiles else None
                    if stages[2] is not None:
                        _emit_back2(nc, pools, yd, stages[2], tf)
                    stages[2] = _emit_back1(nc, pools, stages[1], tf) if stages[1] is not None else None
                    stages[1] = _emit_front2(nc, pools, stages[0], tf) if stages[0] is not None else None
                    stages[0] = nxt
    nc.finalize()
    return nc


_BUILD_CACHE = {}


def _get_nc(shard, tf):
    key = (shard, tf)
    if key not in _BUILD_CACHE:
        _BUILD_CACHE[key] = build_nc(shard, tf)
    return _BUILD_CACHE[key]


def kernel(x: np.ndarray) -> np.ndarray:
    x = np.ascontiguousarray(x, dtype=np.float32)
    n = x.shape[0]
    shard = n // N_CORES
    tf = CFG["tf"]
    while shard % (128 * tf) != 0:
        tf //= 2
    nc = _get_nc(shard, tf)
    in_maps = [{"x": x[i * shard:(i + 1) * shard]} for i in range(N_CORES)]
    res = run_bass_kernel_spmd(nc, in_maps, list(range(N_CORES))).results
    out = np.concatenate([res[i]["y"] for i in range(N_CORES)], axis=0)
    return np.ascontiguousarray(out.astype(np.float32))


# revision 4
# speedup vs baseline: 8.2422x; 8.0991x over previous
"""Nearest-E8 quantizer, v4: rebalanced engines + q-space smalls.

Same math as v3 (see kernel.py docstring), restructured:
  - parities as sign-multiplies: q0 = (-1)^parity(Sr) via bit0<<31 trick,
    qx from xor-reduce bit31, q1 = q0*qx on Pool.
  - margin m = (2-Sa) + e1*(1-q1) - e0*(1-q0), e0 = 1/2-ma+eps, e1 = mina+eps
  - branch shift Gh = c0 ? p0*e0 : -p1*e1 ; Hh = c1/2
  - y = round(x + sign(d0)*Gh - Hh) + Hh, bf16 out
Engine assignment is table-driven; lumpy ops can alternate engines by tile
parity for fractional balance.
"""

import numpy as np

from concourse import bacc
import concourse.mybir as mybir
from concourse.alu_op_type import AluOpType as op
from concourse.bass_utils import run_bass_kernel_spmd
from concourse.tile import TileContext

N_POINTS = 8388608
N_CORES = 8
SHARD = N_POINTS // N_CORES

MAGIC = 12582912.0
EPS = float(2.0 ** -20)
F32 = mybir.dt.float32
BF16 = mybir.dt.bfloat16
U32 = mybir.dt.uint32
X = mybir.AxisListType.X
CP = mybir.ActivationFunctionType.Copy

# engine per op; value "v"=DVE, "g"=Pool, "a"=ACT; tuple = alternate by tile parity
CFG = {
    "tf": 256,
    "bufs": 4,
    "par": "sr",       # "sr": add-reduce r0 + parity-extract; "ta": xor-reduce (x+M)
    "r0": "v",         # ts round
    "d0": "g",         # TT sub
    "v": "g",          # TT add (in-place over xt)
    "u": "g",          # TT bcast sub (in-place)
    "w": "v",          # ts 2-op round (in-place)
    "y": "g",          # +H bcast, bf16 out
    "q1": "v",         # small q0*qx   (Pool smalls are ~1.7us on HW: keep on DVE)
    "bb": "v",         # smalls b1,b2 (e*q mults)
    "cc": "v",         # smalls c1t,c2t (subs)
    "mm": "v",         # smalls m-chain (2 TTs)
    "merge": False,    # fuse ta|d0 xor-reduce, b1|b2, c1t|c2t into single ops
    "skip": (),        # timing-only ablations: subsets of {"red","tail","smalls"}
}
ENGINES = CFG  # back-compat for test.py


def _imm_u32(v):
    return mybir.ImmediateValue(dtype=U32, value=v)


def _ts_u32(eng, out, in0, s0, s1, op0, op1):
    return eng.add_instruction(
        mybir.InstTensorScalarPtr(
            name=eng.bass.get_next_instruction_name(),
            op0=op0, op1=op1,
            ins=[eng.lower_ap(in0), _imm_u32(s0), _imm_u32(s1)],
            outs=[eng.lower_ap(out)],
        )
    )


def _stt_u32(eng, out, in0, scalar_int, in1, op0, op1):
    return eng.add_instruction(
        mybir.InstTensorScalarPtr(
            name=eng.bass.get_next_instruction_name(),
            is_scalar_tensor_tensor=True,
            op0=op0, op1=op1,
            ins=[eng.lower_ap(in0), _imm_u32(scalar_int), eng.lower_ap(in1)],
            outs=[eng.lower_ap(out)],
        )
    )


def _eng(nc, key, t):
    e = CFG[key]
    if isinstance(e, tuple):
        e = e[t % len(e)]
    return {"v": nc.vector, "g": nc.gpsimd, "a": nc.scalar}[e]


def _emit_front1(nc, pools, xd, t, tf):
    P = 128
    pts = P * tf
    FE = tf * 8
    stream, work, small = pools
    s = t * pts
    xt = stream.tile([P, FE], F32, tag="xt")
    nc.sync.dma_start(out=xt[:], in_=xd[s:s + pts, :].rearrange("(p f) c -> p (f c)", p=P))
    re = CFG["r0"]
    re_t = re[t % len(re)] if isinstance(re, tuple) else re
    if CFG["par"] == "ta" and CFG.get("merge"):
        # one [P, 2FE] tile: [ta | d0]; r0 transient in d0's half
        big = work.tile([P, 2 * FE], F32, tag="big")
        ta, d0 = big[:, :FE], big[:, FE:]
        e = {"v": nc.vector, "g": nc.gpsimd}[re_t if re_t != "a" else "v"]
        e.tensor_scalar(ta, xt[:], MAGIC, None, op0=op.add)
        e.tensor_scalar(d0, ta, MAGIC, None, op0=op.subtract)  # r0 in d0 slot
        _eng(nc, "d0", t).tensor_tensor(d0, xt[:], d0, op.subtract)
        return dict(t=t, xt=xt, big=big, r0=None, d0v=d0)
    r0 = work.tile([P, FE], F32, tag="r0")
    d0 = work.tile([P, FE], F32, tag="d0")
    ta = None
    if CFG["par"] == "ta":
        # tA = x + MAGIC (bit0 = parity of round(x)); r0 = tA - MAGIC
        ta = work.tile([P, FE], F32, tag="ta")
        if re_t == "a":
            nc.scalar.activation(ta[:], xt[:], CP, bias=MAGIC)
            nc.scalar.activation(r0[:], ta[:], CP, bias=-MAGIC)
        else:
            e = {"v": nc.vector, "g": nc.gpsimd}[re_t]
            e.tensor_scalar(ta[:], xt[:], MAGIC, None, op0=op.add)
            e.tensor_scalar(r0[:], ta[:], MAGIC, None, op0=op.subtract)
    else:
        if re_t == "a":
            nc.scalar.activation(r0[:], xt[:], CP, bias=MAGIC)
            nc.scalar.activation(r0[:], r0[:], CP, bias=-MAGIC)
        else:
            e = {"v": nc.vector, "g": nc.gpsimd}[re_t]
            e.tensor_scalar(r0[:], xt[:], MAGIC, MAGIC, op0=op.add, op1=op.subtract)
    _eng(nc, "d0", t).tensor_tensor(d0[:], xt[:], r0[:], op.subtract)
    return dict(t=t, xt=xt, r0=r0, d0=d0, ta=ta, d0v=None)


def _emit_front2(nc, pools, st, tf):
    """Reduces + q-space smalls."""
    P = 128
    stream, work, small = pools
    t = st["t"]
    V, G, A = nc.vector, nc.gpsimd, nc.scalar
    Eq = {"v": V, "g": G}

    if CFG["par"] == "ta" and CFG.get("merge"):
        return _emit_front2_merged(nc, pools, st, tf)

    r0, d0 = st["r0"], st["d0"]
    d03 = d0[:].rearrange("p (t c) -> p t c", c=8)
    r03 = r0[:].rearrange("p (t c) -> p t c", c=8)
    d0u3 = d0[:].bitcast(U32).rearrange("p (t c) -> p t c", c=8)

    NS = 10
    arena = small.tile([P, NS * tf], F32, tag="arena")
    sl = lambda i: arena[:, i * tf:(i + 1) * tf]
    Sr, xr, Sa, ma, mina = sl(0), sl(1), sl(2), sl(3), sl(4)
    q1, e0, e1, Gh, Hh = sl(5), sl(6), sl(7), sl(8), sl(9)

    if "red" not in CFG["skip"]:
        if CFG["par"] == "ta":
            ta3 = st["ta"][:].bitcast(U32).rearrange("p (t c) -> p t c", c=8)
            V.tensor_reduce(Sr.bitcast(U32), ta3, axis=X, op=op.bitwise_xor)
        else:
            V.tensor_reduce(Sr, r03, axis=X, op=op.add)
        V.tensor_reduce(xr.bitcast(U32), d0u3, axis=X, op=op.bitwise_xor)
        V.tensor_reduce(Sa, d03, axis=X, op=op.add, apply_absolute_value=True)
        V.tensor_reduce(ma, d03, axis=X, op=op.max, apply_absolute_value=True)
        V.tensor_reduce(mina, d03, axis=X, op=op.min, apply_absolute_value=True)

    if "smalls" in CFG["skip"]:
        st["Gh"] = Gh
        st["Hh"] = Hh
        return st
    # q0 = (-1)^parity(sum r0): from xor(tA) bit0, or from (Sr+MAGIC) bit0
    q0 = Sr  # in-place chain over Sr's slot
    if CFG["par"] != "ta":
        V.tensor_scalar(q0, Sr, MAGIC, None, op0=op.add)
    _ts_u32(V, q0.bitcast(U32), q0.bitcast(U32), 31, 0x3F800000,
            op.logical_shift_left, op.bitwise_xor)
    # qx = (-1)^parity(#neg): bit31 of xor-reduce
    qx = xr
    _ts_u32(V, qx.bitcast(U32), xr.bitcast(U32), 0x80000000, 0x3F800000,
            op.bitwise_and, op.bitwise_xor)
    # q1 = q0*qx
    Eq[CFG["q1"]].tensor_tensor(q1, q0, qx, op.mult)
    # e0 = 1/2 - ma + eps ; e1 = mina + eps ; s2 = 2 - Sa (into Sa's slot)
    A.activation(e0, ma, CP, bias=0.5 + EPS, scale=-1.0)
    A.activation(e1, mina, CP, bias=EPS)
    s2 = Sa
    A.activation(s2, Sa, CP, bias=2.0, scale=-1.0)
    # b1 = e1*q1, b2 = e0*q0 (Pool); c1t = e1-b1 = 2*p1*e1, c2t = e0-b2 = 2*p0*e0
    b1, b2 = ma, mina  # reuse slots (ma/mina consumed by e0/e1)
    Eb = Eq[CFG["bb"]]
    Eb.tensor_tensor(b1, e1, q1, op.mult)
    Eb.tensor_tensor(b2, e0, q0, op.mult)
    c1t, c2t = b1, b2
    Ec = Eq[CFG["cc"]]
    Ec.tensor_tensor(c1t, e1, b1, op.subtract)
    Ec.tensor_tensor(c2t, e0, b2, op.subtract)
    # Gh candidates BEFORE m clobbers anything: Gh0 = c2t/2, g1 = -c1t/2
    A.activation(Gh, c2t, CP, scale=0.5)
    g1 = q1  # q1 slot dead after b1
    A.activation(g1, c1t, CP, scale=-0.5)
    # m = (s2 - c2t) + c1t  (into s2's slot); c0f = m >= 0 (in place)
    m = s2
    Em = Eq[CFG["mm"]]
    Em.tensor_tensor(m, s2, c2t, op.subtract)
    Em.tensor_tensor(m, m, c1t, op.add)
    c0f = m
    V.tensor_scalar(c0f, m, 0.0, None, op0=op.is_ge)
    # Hh = 0.5*(1-c0f); then Gh = branch1 ? g1 : Gh0 (Hh doubles as the mask)
    A.activation(Hh, c0f, CP, bias=0.5, scale=-0.5)
    V.copy_predicated(Gh, Hh.bitcast(U32), g1)

    st["Gh"] = Gh
    st["Hh"] = Hh
    return st


def _emit_front2_merged(nc, pools, st, tf):
    """Merged variant: one xor-reduce over [ta|d0]; fused b/c smalls.

    Arena slots: 0 q0raw->q0, 1 xraw->qx->q1, 2 Sa->s2->m->c0f,
    3 ma->b2->c2t, 4 mina->b1->c1t, 5 e0, 6 e1, 7 tmp, 8 g1, 9 Gh, 10 Hh.
    Pairings: b-mult out (3,4) = in0 (5,6)=(e0|e1) * in1 (0,1)=(q0|q1);
    c-sub out (3,4) = in0 (5,6) - in1 (3,4).
    => slot3 = e0*q0 = b2, c2t = e0-b2 ; slot4 = e1*q1 = b1, c1t = e1-b1.
    """
    P = 128
    stream, work, small = pools
    t = st["t"]
    V, G, A = nc.vector, nc.gpsimd, nc.scalar
    Eq = {"v": V, "g": G}
    big = st["big"]
    FE = tf * 8
    d0 = st["d0v"]
    d03 = d0.rearrange("p (t c) -> p t c", c=8)

    NS = 11
    arena = small.tile([P, NS * tf], F32, tag="arena")
    sl = lambda i: arena[:, i * tf:(i + 1) * tf]

    big4 = big[:].bitcast(U32).rearrange("p (k t c) -> p k t c", k=2, c=8)
    xo2 = arena[:, 0:2 * tf].bitcast(U32).rearrange("p (k t) -> p k t", k=2)
    V.tensor_reduce(xo2, big4, axis=X, op=op.bitwise_xor)
    Sa, ma, mina = sl(2), sl(3), sl(4)
    V.tensor_reduce(Sa, d03, axis=X, op=op.add, apply_absolute_value=True)
    V.tensor_reduce(ma, d03, axis=X, op=op.max, apply_absolute_value=True)
    V.tensor_reduce(mina, d03, axis=X, op=op.min, apply_absolute_value=True)

    q0, qx = sl(0), sl(1)
    _ts_u32(V, q0.bitcast(U32), q0.bitcast(U32), 31, 0x3F800000,
            op.logical_shift_left, op.bitwise_xor)
    _ts_u32(V, qx.bitcast(U32), qx.bitcast(U32), 0x80000000, 0x3F800000,
            op.bitwise_and, op.bitwise_xor)
    q1 = qx  # in-place over qx -> (q0|q1) adjacent
    Eq[CFG["q1"]].tensor_tensor(q1, q0, qx, op.mult)

    e0, e1 = sl(5), sl(6)
    A.activation(e0, ma, CP, bias=0.5 + EPS, scale=-1.0)
    A.activation(e1, mina, CP, bias=EPS)
    s2 = Sa
    A.activation(s2, Sa, CP, bias=2.0, scale=-1.0)

    # fused b-mult: (3,4) = (e0|e1) * (q0|q1) ; fused c-sub: (3,4) = (e0|e1) - (3,4)
    ee = arena[:, 5 * tf:7 * tf]
    qq = arena[:, 0:2 * tf]
    bb = arena[:, 3 * tf:5 * tf]
    Eq[CFG["bb"]].tensor_tensor(bb, ee, qq, op.mult)
    Eq[CFG["cc"]].tensor_tensor(bb, ee, bb, op.subtract)
    c2t, c1t = sl(3), sl(4)

    Gh, Hh, g1 = sl(9), sl(10), sl(8)
    A.activation(Gh, c2t, CP, scale=0.5)
    A.activation(g1, c1t, CP, scale=-0.5)

    m = s2
    Em = Eq[CFG["mm"]]
    Em.tensor_tensor(m, s2, c2t, op.subtract)
    Em.tensor_tensor(m, m, c1t, op.add)
    c0f = m
    V.tensor_scalar(c0f, m, 0.0, None, op0=op.is_ge)
    A.activation(Hh, c0f, CP, bias=0.5, scale=-0.5)
    V.copy_predicated(Gh, Hh.bitcast(U32), g1)

    st["Gh"] = Gh
    st["Hh"] = Hh
    return st


def _emit_back1(nc, pools, st, tf):
    P = 128
    t = st["t"]
    xt, Gh = st["xt"], st["Gh"]
    if "tail" in CFG["skip"]:
        return st
    d0ap = st["d0v"] if st.get("d0v") is not None else st["d0"][:]
    d0u3 = d0ap.bitcast(U32).rearrange("p (t c) -> p t c", c=8)
    Gh_b = Gh.bitcast(U32).unsqueeze(2).broadcast_to([P, tf, 8])
    # g = (d0 & sign) ^ Gh  (in place over d0)  [DVE only: stt]
    _stt_u32(nc.vector, d0u3, d0u3, 0x80000000, Gh_b, op.bitwise_and, op.bitwise_xor)
    # v = x + g (in place over xt)
    _eng(nc, "v", t).tensor_tensor(xt[:], xt[:], d0ap, op.add)
    return st


def _emit_back2(nc, pools, yd, st, tf):
    P = 128
    pts = P * tf
    FE = tf * 8
    stream, work, small = pools
    t = st["t"]
    xt, Hh = st["xt"], st["Hh"]
    s = t * pts
    y_rows = yd[s:s + pts, :].rearrange("(p f) c -> p (f c)", p=P)
    H_b = Hh.unsqueeze(2).broadcast_to([P, tf, 8])
    xt3 = xt[:].rearrange("p (t c) -> p t c", c=8)
    if "tail" in CFG["skip"]:
        yt = stream.tile([P, FE], BF16, tag="yt")
        nc.vector.tensor_scalar(yt[:], xt[:], 1.0, None, op0=op.mult)
        nc.sync.dma_start(out=y_rows, in_=yt[:])
        return
    # u = v - H (in place)
    _eng(nc, "u", t).tensor_tensor(xt3, xt3, H_b, op.subtract)
    # w = round(u) (in place)
    we = CFG["w"]
    if we == "a":
        nc.scalar.activation(xt[:], xt[:], CP, bias=MAGIC)
        nc.scalar.activation(xt[:], xt[:], CP, bias=-MAGIC)
    else:
        _eng(nc, "w", t).tensor_scalar(xt[:], xt[:], MAGIC, MAGIC, op0=op.add, op1=op.subtract)
    # y = w + H -> bf16
    yt = stream.tile([P, FE], BF16, tag="yt")
    yt3 = yt[:].rearrange("p (t c) -> p t c", c=8)
    ye = CFG["y"]
    ye_t = ye[t % len(ye)] if isinstance(ye, tuple) else ye
    if ye_t == "v":
        nc.vector.scalar_tensor_tensor(yt3, H_b, 1.0, xt3, op0=op.mult, op1=op.add)
    else:
        nc.gpsimd.tensor_tensor(yt3, xt3, H_b, op.add)
    nc.sync.dma_start(out=y_rows, in_=yt[:])


def build_nc(shard=SHARD, tf=None, reps=1, cfg=None):
    if cfg:
        CFG.update(cfg)
    P = 128
    tf = tf or CFG["tf"]
    pts = P * tf
    assert shard % pts == 0
    ntiles = shard // pts

    nc = bacc.Bacc("TRN2", target_bir_lowering=False, debug=False, num_devices=N_CORES)
    xd = nc.declare_dram_parameter("x", [shard, 8], F32, isOutput=False)
    yd = nc.declare_dram_parameter("y", [shard, 8], BF16, isOutput=True)

    with TileContext(nc) as tc:
        with (
            tc.tile_pool(name="stream", bufs=CFG["bufs"]) as stream,
            tc.tile_pool(name="work", bufs=CFG["bufs"]) as work,
            tc.tile_pool(name="small", bufs=CFG["bufs"]) as small,
        ):
            for _ in range(reps):
                pools = (stream, work, small)
                stages = [None, None, None]
                for t in range(ntiles + 3):
                    nxt = _emit_front1(nc, pools, xd, t, tf) if t < ntiles else None
                    if stages[2] is not None:
                        _emit_back2(nc, pools, yd, stages[2], tf)
                    stages[2] = _emit_back1(nc, pools, stages[1], tf) if stages[1] is not None else None
                    stages[1] = _emit_front2(nc, pools, stages[0], tf) if stages[0] is not None else None
                    stages[0] = nxt
    nc.finalize()
    return nc


_BUILD_CACHE = {}


def _get_nc(shard, tf):
    key = (shard, tf)
    if key not in _BUILD_CACHE:
        _BUILD_CACHE[key] = build_nc(shard, tf)
    return _BUILD_CACHE[key]


def kernel(x: np.ndarray) -> np.ndarray:
    x = np.ascontiguousarray(x, dtype=np.float32)
    n = x.shape[0]
    shard = n // N_CORES
    tf = CFG["tf"]
    while shard % (128 * tf) != 0:
        tf //= 2
    nc = _get_nc(shard, tf)
    in_maps = [{"x": x[i * shard:(i + 1) * shard]} for i in range(N_CORES)]
    res = run_bass_kernel_spmd(nc, in_maps, list(range(N_CORES))).results
    out = np.concatenate([res[i]["y"] for i in range(N_CORES)], axis=0)
    return np.ascontiguousarray(out.astype(np.float32))
